# revision 2
# baseline (speedup 1.0000x reference)
"""Trainium2 Bass kernel for nn_ContactPredictionHead.

Computes, for inputs (B=16, P=87, K=4, N=32768, D=256, H=128):
  human_probs  = sigmoid(relu(hq @ W1 + b1) @ W2 + b2)      (B, 87)
  human_logits = (the pre-sigmoid logits)                    (B, 87)
  object_coords = softmax((oq @ Wq + bq) @ feats^T / 8) @ xyz  (B, 4, 3)

Sharding: data-parallel over B across 8 NeuronCores (2 batches/core).

Per-core design (per batch, pipelined over 64 n-blocks of 512 points):
  - DMA feats block [512, 256] -> SBUF natural layout [128p, 4t, 256d]
  - PE transposes each [128n, 128d] chunk -> PSUM [128d, 128n] (fp32 exact)
  - DVE copies transposed chunks to SBUF (FT)
  - logits^T [n, 4] = FT.T @ qT accumulated over the two 128-d chunks,
    8 n-chunks share one PSUM bank at different free offsets
  - ACT: E = exp(logits * 1/8)  (softmax max-subtraction is skipped: logits
    have std ~1.2, max < ~7 over 32k samples, exp stays well inside fp32)
  - coords+sumexp fused: one PSUM [4, 4] accumulates E.T @ [1|xyz] over all
    n-chunks of the batch; final divide by the ones-column normalizes.
"""

import numpy as np
from contextlib import ExitStack

import concourse.bass as bass
import concourse.bacc as bacc
import concourse.tile as tile
from concourse import mybir
from concourse.bass_utils import run_bass_kernel_spmd
from concourse.masks import make_identity

F32 = mybir.dt.float32

N_CORES = 8
B_FULL = 16
BPC = B_FULL // N_CORES  # batches per core
P_Q = 87
K_Q = 4
D = 256
HID = 128
N_FULL = 32768

NBLK = 512            # points per pipeline block
T_PER_BLK = NBLK // 128   # 4 chunks of 128 points per block
DC = D // 128             # 2 contraction chunks


def build_program(n_points=N_FULL, batches=BPC):
    nblocks = n_points // NBLK
    nt_total = n_points // 128

    nc = bacc.Bacc("TRN2", target_bir_lowering=False, debug=False,
                   num_devices=N_CORES)

    # Per-core shards (SPMD: same program, different data per core)
    hq_d = nc.dram_tensor("hq", [batches, P_Q, D], F32, kind="ExternalInput").ap()
    oq_d = nc.dram_tensor("oq", [batches, K_Q, D], F32, kind="ExternalInput").ap()
    feats_d = nc.dram_tensor("feats", [batches, n_points, D], F32,
                             kind="ExternalInput").ap()
    xyz_d = nc.dram_tensor("xyz", [batches, n_points, 3], F32,
                           kind="ExternalInput").ap()
    w1_d = nc.dram_tensor("W1", [D, HID], F32, kind="ExternalInput").ap()
    b1_d = nc.dram_tensor("b1", [HID], F32, kind="ExternalInput").ap()
    w2_d = nc.dram_tensor("W2", [HID, 1], F32, kind="ExternalInput").ap()
    b2_d = nc.dram_tensor("b2", [1], F32, kind="ExternalInput").ap()
    wq_d = nc.dram_tensor("Wq", [D, D], F32, kind="ExternalInput").ap()
    bq_d = nc.dram_tensor("bq", [D], F32, kind="ExternalInput").ap()

    probs_d = nc.dram_tensor("human_probs", [batches, P_Q], F32,
                             kind="ExternalOutput").ap()
    hlog_d = nc.dram_tensor("human_logits", [batches, P_Q], F32,
                            kind="ExternalOutput").ap()
    coords_d = nc.dram_tensor("object_coords", [batches, K_Q, 3], F32,
                              kind="ExternalOutput").ap()

    with tile.TileContext(nc) as tc:
        with ExitStack() as ctx:
            consts = ctx.enter_context(tc.tile_pool(name="consts", bufs=1))
            fpool = ctx.enter_context(tc.tile_pool(name="feats_in", bufs=3))
            ftpool = ctx.enter_context(tc.tile_pool(name="ft", bufs=10))
            epool = ctx.enter_context(tc.tile_pool(name="exp", bufs=3))
            bpool = ctx.enter_context(tc.tile_pool(name="perbatch", bufs=2))
            smallp = ctx.enter_context(tc.tile_pool(name="small", bufs=2))
            # PSUM: every tile takes a full bank; 8 banks total.
            tppool = ctx.enter_context(
                tc.tile_pool(name="tp", bufs=3, space="PSUM"))
            lppool = ctx.enter_context(
                tc.tile_pool(name="lp", bufs=2, space="PSUM"))
            cppool = ctx.enter_context(
                tc.tile_pool(name="cp", bufs=1, space="PSUM"))
            sppool = ctx.enter_context(
                tc.tile_pool(name="sp", bufs=2, space="PSUM"))

            # ---- constants ----
            ident = consts.tile([128, 128], F32)
            make_identity(nc, ident)
            ones_r = consts.tile([1, 128], F32)   # row of ones (bias bcast)
            nc.gpsimd.memset(ones_r[:], 1.0)

            w1 = consts.tile([128, DC, HID], F32)
            nc.sync.dma_start(w1[:], w1_d.rearrange("(c p) h -> p c h", p=128))
            b1r = consts.tile([1, HID], F32)
            nc.sync.dma_start(b1r[:], b1_d[None, :])
            w2 = consts.tile([HID, 1], F32)
            nc.sync.dma_start(w2[:], w2_d[:, :])
            b2s = consts.tile([1, 1], F32)
            nc.sync.dma_start(b2s[:], b2_d[None, :])
            wq = consts.tile([128, DC, DC, 128], F32)
            nc.sync.dma_start(
                wq[:], wq_d.rearrange("(ci p) (cj o) -> p ci cj o", p=128, o=128))
            bqr = consts.tile([1, D], F32)
            nc.sync.dma_start(bqr[:], bq_d[None, :])

            for b in range(batches):
                # ---- q projection: qT[do, k] = Wq^T oq[b]^T + bq ----
                oqn = smallp.tile([K_Q, DC, 128], F32, tag="oqn")
                nc.sync.dma_start(
                    oqn[:], oq_d[b].rearrange("k (c d) -> k c d", c=DC))
                oqT = smallp.tile([128, DC, K_Q], F32, tag="oqT")
                for c in range(DC):
                    tp = sppool.tile([128, K_Q], F32, tag="sp")
                    nc.tensor.transpose(tp[:], oqn[:, c, :], ident[:K_Q, :K_Q])
                    nc.vector.tensor_copy(oqT[:, c, :], tp[:])
                qT = bpool.tile([128, DC, K_Q], F32, tag="qT")
                for cj in range(DC):
                    qp = sppool.tile([128, K_Q], F32, tag="sp")
                    for ci in range(DC):
                        nc.tensor.matmul(qp[:], wq[:, ci, cj, :], oqT[:, ci, :],
                                         start=(ci == 0), stop=False)
                    nc.tensor.matmul(qp[:], bqr[:, cj * 128:(cj + 1) * 128],
                                     ones_r[:, :K_Q], start=False, stop=True)
                    nc.vector.tensor_copy(qT[:, cj, :], qp[:])

                # ---- human head ----
                hqn = smallp.tile([P_Q, DC, 128], F32, tag="hqn")
                nc.sync.dma_start(
                    hqn[:], hq_d[b].rearrange("k (c d) -> k c d", c=DC))
                hqT = smallp.tile([128, DC, P_Q], F32, tag="hqT")
                for c in range(DC):
                    tp = sppool.tile([128, P_Q], F32, tag="sp")
                    nc.tensor.transpose(tp[:], hqn[:, c, :], ident[:P_Q, :P_Q])
                    nc.vector.tensor_copy(hqT[:, c, :], tp[:])
                hp = sppool.tile([HID, P_Q], F32, tag="sp")
                for c in range(DC):
                    nc.tensor.matmul(hp[:], w1[:, c, :], hqT[:, c, :],
                                     start=(c == 0), stop=False)
                nc.tensor.matmul(hp[:], b1r[:], ones_r[:, :P_Q],
                                 start=False, stop=True)
                hrelu = smallp.tile([HID, P_Q], F32, tag="hrelu")
                nc.scalar.activation(hrelu[:], hp[:],
                                     mybir.ActivationFunctionType.Relu)
                lg = sppool.tile([P_Q, 1], F32, tag="sp")
                nc.tensor.matmul(lg[:], hrelu[:], w2[:], start=True, stop=False)
                nc.tensor.matmul(lg[:], ones_r[:, :P_Q], b2s[:],
                                 start=False, stop=True)
                hlog = smallp.tile([P_Q, 1], F32, tag="hlog")
                nc.scalar.activation(hlog[:], lg[:],
                                     mybir.ActivationFunctionType.Copy)
                hprob = smallp.tile([P_Q, 1], F32, tag="hprob")
                nc.scalar.activation(hprob[:], lg[:],
                                     mybir.ActivationFunctionType.Sigmoid)
                nc.sync.dma_start(hlog_d[b], hlog[:, 0])
                nc.sync.dma_start(probs_d[b], hprob[:, 0])

                # ---- xyz with ones column: [128, nt, 4] = [1 | x y z] ----
                xyzo = bpool.tile([128, nt_total, 4], F32, tag="xyzo")
                nc.gpsimd.memset(xyzo[:, :, 0:1], 1.0)
                nc.sync.dma_start(
                    xyzo[:, :, 1:4], xyz_d[b].rearrange("(t p) c -> p t c", p=128))

                # ---- attention pooling main loop ----
                cps = cppool.tile([K_Q, 4], F32, tag="coords_ps")
                for nb in range(nblocks):
                    fblk = fpool.tile([128, T_PER_BLK, D], F32, tag="fblk")
                    nc.sync.dma_start(
                        fblk[:],
                        feats_d[b, nb * NBLK:(nb + 1) * NBLK, :]
                        .rearrange("(t p) d -> p t d", p=128))
                    lps = lppool.tile([128, 4 * T_PER_BLK], F32, tag="lps")
                    for t in range(T_PER_BLK):
                        for c in range(DC):
                            tps = tppool.tile([128, 128], F32, tag="tps")
                            nc.tensor.transpose(
                                tps[:], fblk[:, t, c * 128:(c + 1) * 128],
                                ident[:])
                            ft = ftpool.tile([128, 128], F32, tag="ft")
                            nc.vector.tensor_copy(ft[:], tps[:])
                            nc.tensor.matmul(lps[:, 4 * t:4 * t + 4], ft[:],
                                             qT[:, c, :],
                                             start=(c == 0), stop=(c == 1))
                    ee = epool.tile([128, 4 * T_PER_BLK], F32, tag="ee")
                    nc.scalar.activation(ee[:], lps[:],
                                         mybir.ActivationFunctionType.Exp,
                                         scale=0.125)
                    for t in range(T_PER_BLK):
                        nt = nb * T_PER_BLK + t
                        nc.tensor.matmul(cps[:], ee[:, 4 * t:4 * t + 4],
                                         xyzo[:, nt, :],
                                         start=(nt == 0),
                                         stop=(nt == nt_total - 1))

                # ---- normalize coords ----
                rcp = smallp.tile([K_Q, 1], F32, tag="rcp")
                nc.vector.reciprocal(rcp[:], cps[:, 0:1])
                cvals = smallp.tile([K_Q, 3], F32, tag="cvals")
                nc.vector.tensor_scalar_mul(cvals[:], cps[:, 1:4], rcp[:])
                nc.sync.dma_start(coords_d[b], cvals[:])

    nc.compile()
    return nc


_PROGRAM = None


def _get_program():
    global _PROGRAM
    if _PROGRAM is None:
        _PROGRAM = build_program()
    return _PROGRAM


def make_in_maps(human_queries, object_queries, object_feats, object_xyz,
                 W1, b1, W2, b2, Wq, bq):
    asf = lambda x: np.ascontiguousarray(np.asarray(x, dtype=np.float32))
    hq, oq, feats, xyz = asf(human_queries), asf(object_queries), \
        asf(object_feats), asf(object_xyz)
    W1, b1, W2, b2, Wq, bq = map(asf, (W1, b1, W2, b2, Wq, bq))
    in_maps = []
    for c in range(N_CORES):
        s = slice(c * BPC, (c + 1) * BPC)
        in_maps.append({
            "hq": hq[s], "oq": oq[s], "feats": feats[s], "xyz": xyz[s],
            "W1": W1, "b1": b1, "W2": W2, "b2": b2, "Wq": Wq, "bq": bq,
        })
    return in_maps


def assemble(results):
    probs = np.concatenate([r["human_probs"] for r in results], axis=0)
    hlog = np.concatenate([r["human_logits"] for r in results], axis=0)
    coords = np.concatenate([r["object_coords"] for r in results], axis=0)
    return probs, hlog, coords


def kernel(human_queries, object_queries, object_feats, object_xyz,
           W1, b1, W2, b2, Wq, bq):
    nc = _get_program()
    in_maps = make_in_maps(human_queries, object_queries, object_feats,
                           object_xyz, W1, b1, W2, b2, Wq, bq)
    res = run_bass_kernel_spmd(nc, in_maps, list(range(N_CORES)))
    return assemble(res.results)
